# revision 39
# baseline (speedup 1.0000x reference)
"""Multi-head attention (B=2, S=2048, D=1024, H=16) on 8 trn2 NeuronCores.

Sharding: data-parallel over batch (2) x tensor-parallel over head-groups (4).
Core c handles batch b = c // 4 and heads [4g, 4g+4) with g = c % 4.

Per core (all matmul operand paths bf16; PSUM accumulation fp32):
- Inputs stream in over FOUR parallel DMA queues (sync/vector/scalar/
  gpsimd issue rings) so the startup transfer wall is the largest single
  transfer, not the sum; q/k weights are split ct0/ct1 so the score
  stream's gating transfer is 0.25 MB.
- q/k are produced in transposed layout [ch, S]; v in [S, ch] layout with an
  appended ones column per head (softmax denominator rides the attnV matmul).
- Per head, scores^T = k_h @ q_h^T on the PE (K=64). Each kt-tile's two
  512-wide score matmuls are issued as a row-group pair (the second operand
  pair comes from partition-swapped copies of q/k), so both run concurrently
  in disjoint PE row groups. The first 4 units of pass 0 run the second
  half serially instead (their swapped dups don't exist yet), which lets
  the exp stream start after just one q chunk + one k chunk.
- exp on the scalar engine (1/8 scale folded in; scores are ~N(0,1) so no
  max-subtraction is needed), writing bf16 directly to SBUF.
- Attention output via the ones-augmented V (M=65) accumulated in PSUM per
  q-pass; numerators+denominator are evacuated to SBUF in bf16 (frees the
  PSUM bank early), the 1024 denominators are reshaped to [16,64] via
  SBUF-to-SBUF DMA so the DVE reciprocal uses 16 lanes instead of 1, then
  a broadcast (DRAM-bounce stride-0 normally; a K=32 PE matmul into free
  PSUM for the final tail pass) + DVE multiply produce attnT in bf16.
- Row-parallel output projection in bf16; PSUM evacuation of the late
  output tiles is split between the DVE and the (by then idle) scalar
  engine. The host upcasts and sums the 8 partials and adds the bias.

Scheduling: the attention phase is ACT(exp)-bound. All 8 (head, q-pass)
passes run as ONE continuous 128-unit stream (scores/exp DEPTH units ahead
of the in-order attnV stream) so head boundaries cost no pipeline drain.
Every projection/output tile rides inside the stream as a PE filler with
an explicit per-kt pop schedule tuned to its operand-arrival deadline.
"""

import numpy as np

import concourse.bass as bass
import concourse.tile as tile
from concourse import bacc, mybir
from concourse.bass_utils import run_bass_kernel_spmd

P = 128
S = 2048          # sequence length
D = 1024          # model dim
HD = 64           # head dim
HL = 4            # heads per core
CL = HL * HD      # 256 local channels
DC = D // P       # 8 contraction chunks
ST = S // P       # 16 seq tiles
QH = 1024         # q-half width
NCORES = 8
GROUPS = 4

f32 = mybir.dt.float32
bf16 = mybir.dt.bfloat16
EXP_BUFS = 14
OB_BUFS = 6
DEPTH = 10
FT = mybir.ActivationFunctionType

_CACHE = {}


def _attn_stream(nc, pools, passes, v_aug, rb_drams):
    """Emit all passes as one continuous unit stream. Each pass dict:
    qh/kh/qx/kx (operand row-slices), attnT, h, co, qp, serial_until
    (units below this kt emit the j=1 score matmul without row-group
    pairing), spops/apops (kt -> closure, popped in the scores / attnV
    stream), last (tail pass: PE-broadcast normalize)."""
    psp, expp, rbp, nump = pools["ps"], pools["exp"], pools["rb"], pools["num"]
    N = len(passes) * ST
    exs = {}

    def emit_scores_exp(i):
        c = passes[i // ST]
        kt = i % ST
        f = c["spops"].pop(kt, None)
        if f is not None:
            f()
        ps = psp.tile([P, QH], f32, tag="ps", name="ps")
        qoff = c["qp"] * QH
        serial = kt < c["serial_until"]
        for j in range(2):
            kt_src = c["kh"] if (j == 0 or serial) else c["kx"]
            qt_src = c["qh"] if (j == 0 or serial) else c["qx"]
            nc.tensor.matmul(
                ps[:, j * 512 : (j + 1) * 512],
                kt_src[:, kt * P : (kt + 1) * P],
                qt_src[:, qoff + j * 512 : qoff + (j + 1) * 512],
                start=True,
                stop=True,
                skip_group_check=True,
            )
        ex = expp.tile([P, QH], bf16, tag="exp", name="ex")
        nc.scalar.activation(ex[:], ps[:], FT.Exp, scale=0.125)
        exs[i] = ex

    def emit_attnv(i):
        c = passes[i // ST]
        kt = i % ST
        if kt == 0:
            c["oh"] = psp.tile([HD + 1, QH], f32, tag="ps", name="oh")
        oh = c["oh"]
        ex = exs.pop(i)
        for j in range(2):
            nc.tensor.matmul(
                oh[:, j * 512 : (j + 1) * 512],
                v_aug[:, kt, c["h"], :],
                ex[:, j * 512 : (j + 1) * 512],
                start=(kt == 0),
                stop=(kt == ST - 1),
                skip_group_check=True,
            )
        f = c["apops"].pop(kt, None)
        if f is not None:
            f()
        if kt == ST - 1:
            _normalize(i // ST, c, oh)

    def _normalize(p_idx, c, oh):
        # Evacuate numerators + denominator row to SBUF bf16 (releases the
        # oh PSUM bank after one cast), reshape the 1024 denominators to
        # [16,64] (SBUF->SBUF DMA) so the DVE reciprocal uses 16 lanes,
        # then broadcast to 64 partitions and multiply into attnT (bf16).
        co = c["co"]
        num = nump.tile([HD + 1, QH], bf16, tag="num", name="num")
        nc.vector.tensor_copy(num[:], oh[:])
        r16 = rbp.tile([16, 64], bf16, tag="r16", name="r16")
        r16b = rbp.tile([16, 64], bf16, tag="r16", name="r16b")
        nc.sync.dma_start(r16[:], num[HD : HD + 1, :])
        with nc.allow_low_precision(reason="softmax denom reciprocal bf16"):
            nc.vector.reciprocal(r16b[:], r16[:])
        dst = c["attnT"][co : co + HD, c["qp"] * QH : (c["qp"] + 1) * QH]
        if c["last"]:
            # Tail pass: scores are done, PSUM is free. Broadcast the
            # reciprocal row across 64 partitions with a K=32 PE matmul
            # (ones in row 0, zero pad below) instead of two DRAM hops.
            row_pad = pools["row_pad"]
            nc.sync.dma_start(row_pad[0:1, :], r16b[:])
            psb = psp.tile([HD, QH], f32, tag="ps", name="psb")
            for j in range(2):
                nc.tensor.matmul(
                    psb[:, j * 512 : (j + 1) * 512],
                    pools["ones_col"][:],
                    row_pad[:, j * 512 : (j + 1) * 512],
                    start=True,
                    stop=True,
                    skip_group_check=True,
                )
            nc.vector.tensor_mul(dst, num[0:HD, :], psb[:])
            return
        rbd_b = rb_drams[p_idx]
        w2 = nc.sync.dma_start(
            rbd_b[:].rearrange("a (p f) -> (a p) f", p=16), r16b[:]
        )
        rbt = rbp.tile([HD, QH], bf16, tag="rbt", name="rbt")
        dram_ap = rbd_b[0:1, :]
        bcast_src = bass.AP(
            tensor=dram_ap.tensor,
            offset=dram_ap.offset,
            ap=[[0, HD]] + list(dram_ap.ap[1:]),
        )
        r2 = nc.gpsimd.dma_start(rbt[:], bcast_src)
        tile.add_dep_helper(r2.ins, w2.ins, sync=True, reason="recip bounce")
        nc.vector.tensor_mul(dst, num[0:HD, :], rbt[:])

    for i in range(N + DEPTH):
        if i < N:
            emit_scores_exp(i)
        if i >= DEPTH:
            emit_attnv(i - DEPTH)
    for c in passes:
        assert not c["spops"] and not c["apops"], "unconsumed fillers"


def _build(reps=1):
    nc = bacc.Bacc(None, target_bir_lowering=False)
    xT = nc.dram_tensor("xT", [D, S], bf16, kind="ExternalInput")
    wqT = nc.dram_tensor("wqT", [D, CL], bf16, kind="ExternalInput")
    wkT = nc.dram_tensor("wkT", [D, CL], bf16, kind="ExternalInput")
    wvT = nc.dram_tensor("wvT", [D, CL], bf16, kind="ExternalInput")
    woT = nc.dram_tensor("woT", [CL, D], bf16, kind="ExternalInput")
    outs = [
        nc.dram_tensor(f"outp{j}", [S, D], bf16, kind="ExternalOutput")
        for j in range(2)
    ]

    with tile.TileContext(nc) as tc:
        for rep in range(reps):
            if rep:
                tc.strict_bb_all_engine_barrier()
            _emit_body(nc, tc, xT, wqT, wkT, wvT, woT, outs, rep)
    nc.compile()
    return nc


def _emit_body(nc, tc, xT, wqT, wkT, wvT, woT, outs, rep):
    rb_drams = [
        nc.dram_tensor(f"rbd_{rep}_{i}", [1, QH], bf16, kind="Internal")
        for i in range(8)
    ]
    with (
        tc.tile_pool(name="main", bufs=1) as main,
        tc.tile_pool(name="qk", bufs=2) as qkp,
        tc.tile_pool(name="exp", bufs=EXP_BUFS) as expp,
        tc.tile_pool(name="rb", bufs=2) as rbp,
        tc.tile_pool(name="num", bufs=2) as nump,
        tc.tile_pool(name="ob", bufs=OB_BUFS) as obp,
        tc.tile_pool(name="ps", bufs=3, space="PSUM") as psp,
        tc.tile_pool(name="aux", bufs=2, space="PSUM") as auxp,
    ):
        pools = {"ps": psp, "exp": expp, "rb": rbp, "num": nump}

        v_aug = main.tile([P, ST, HL, HD + 1], bf16)
        nc.vector.memset(v_aug[:, :, :, HD : HD + 1], 1.0)
        # K=32 broadcast operands: stationary has ones in row 0 and zeros
        # below; the moving row-pad is zeroed so uninit rows can't inject
        # NaN (anything x 0 weight must be a real 0 contribution).
        ones_col = main.tile([32, HD], bf16)
        nc.vector.memset(ones_col[:], 0.0)
        nc.vector.memset(ones_col[0:1, :], 1.0)
        row_pad = main.tile([32, QH], bf16)
        nc.vector.memset(row_pad[:], 0.0)
        pools["ones_col"] = ones_col
        pools["row_pad"] = row_pad
        # touch Exp once so the ACT table load happens during the DMA
        # head instead of delaying the first real softmax exp
        actwarm = main.tile([P, 1], f32)
        nc.vector.memset(actwarm[:], 1.0)
        nc.scalar.activation(actwarm[:], actwarm[:], FT.Exp)
        attnT0 = main.tile([P, S], bf16, tag="attnT0")
        attnT1 = main.tile([P, S], bf16, tag="attnT1")
        attnT = [attnT0, attnT1]
        wo_sb = main.tile([P, 2, D], bf16)

        def wo_evac(ob, po, eng):
            # PSUM evacuation: DVE normally; the scalar engine once the
            # exp stream is done (it can read PSUM too).
            if eng == "act":
                nc.scalar.copy(ob[:], po[:])
            else:
                nc.vector.tensor_copy(ob[:], po[:])

        def emit_wo(cc, st, eng="dve"):
            def go():
                ob = obp.tile([P, QH], bf16, tag="obw")
                for j in range(2):
                    po = auxp.tile([P, 512], f32, tag="aux")
                    nc.tensor.matmul(
                        po[:],
                        attnT[cc][:, st * P : (st + 1) * P],
                        wo_sb[:, cc, j * 512 : (j + 1) * 512],
                        start=True,
                        stop=True,
                        skip_group_check=True,
                    )
                    wo_evac(ob[:, j * 512 : (j + 1) * 512], po, eng)
                nc.gpsimd.dma_start(outs[cc][st * P : (st + 1) * P, :], ob[:])
            return go

        def emit_wo_wide(cc, st, eng="dve"):
            # Tail variant: scores are done, borrow a [P, 1024] PSUM tile
            # from the ps pool, one fused evacuation, one output DMA.
            po = psp.tile([P, QH], f32, tag="ps")
            for j in range(2):
                nc.tensor.matmul(
                    po[:, j * 512 : (j + 1) * 512],
                    attnT[cc][:, st * P : (st + 1) * P],
                    wo_sb[:, cc, j * 512 : (j + 1) * 512],
                    start=True,
                    stop=True,
                    skip_group_check=True,
                )
            ob = obp.tile([P, QH], bf16, tag="obw")
            wo_evac(ob, po, eng)
            nc.gpsimd.dma_start(outs[cc][st * P : (st + 1) * P, :], ob[:])

        with tc.tile_pool(name="w", bufs=1) as wp:
            x_sb = wp.tile([P, DC, S], bf16)
            wq_sb = wp.tile([P, DC, CL], bf16)
            wk_sb = wp.tile([P, DC, CL], bf16)
            wv_sb = wp.tile([P, DC, CL], bf16)
            # Parallel DMA head across four issue queues: the stream is
            # gated on x block 0 + wq ct0 + wk ct0 (+ wv for v tiles), so
            # those go on separate queues and the ct1 halves come later.
            xTr = xT[:].rearrange("(c p) s -> p c s", p=P)
            wqTr = wqT[:].rearrange("(c p) m -> p c m", p=P)
            wkTr = wkT[:].rearrange("(c p) m -> p c m", p=P)
            nc.sync.dma_start(x_sb[:, :, 0:256], xTr[:, :, 0:256])
            nc.gpsimd.dma_start(x_sb[:, :, 256:512], xTr[:, :, 256:512])
            nc.scalar.dma_start(
                wv_sb[:], wvT[:].rearrange("(c p) m -> p c m", p=P)
            )
            nc.scalar.dma_start(wq_sb[:, :, 0:P], wqTr[:, :, 0:P])
            nc.scalar.dma_start(wk_sb[:, :, 0:P], wkTr[:, :, 0:P])
            nc.gpsimd.dma_start(x_sb[:, :, 512:1024], xTr[:, :, 512:1024])
            nc.scalar.dma_start(x_sb[:, :, 1024:1536], xTr[:, :, 1024:1536])
            nc.scalar.dma_start(x_sb[:, :, 1536:2048], xTr[:, :, 1536:2048])
            nc.gpsimd.dma_start(wq_sb[:, :, P:CL], wqTr[:, :, P:CL])
            nc.gpsimd.dma_start(wk_sb[:, :, P:CL], wkTr[:, :, P:CL])
            nc.gpsimd.dma_start(
                wo_sb[:], woT[:].rearrange("(c p) d -> p c d", p=P)
            )

            # HAM warm-up: the PE idles through the barrier + DMA head and
            # re-throttles to 1.2 GHz; ~8us of junk matmuls on the zeroed
            # row_pad keep it busy so the pre-stream projections (and the
            # first exp) run at full clock.
            warm = psp.tile([HD, 512], f32, tag="ps", name="warm")
            for _ in range(10):
                nc.tensor.matmul(
                    warm[:],
                    ones_col[:],
                    row_pad[:, 0:512],
                    start=True,
                    stop=True,
                    skip_group_check=True,
                )

            q_tiles, k_tiles, qd_tiles, kd_tiles = [], [], [], []
            for ct in range(2):
                q_tiles.append(qkp.tile([P, S], bf16, tag="q", name=f"q{ct}"))
                k_tiles.append(qkp.tile([P, S], bf16, tag="k", name=f"k{ct}"))
                qd_tiles.append(qkp.tile([P, S], bf16, tag="qd", name=f"qd{ct}"))
                kd_tiles.append(qkp.tile([P, S], bf16, tag="kd", name=f"kd{ct}"))

            def proj_chunk(w_sb, dst, dst_d, ct, nch, pool, dup=True):
                # q/k projection chunk: [P, 512] of the transposed q/k,
                # plus (optionally) the partition-swapped dup used by the
                # row-group-paired j=1 score matmuls. qd is only ever read
                # at chunks n1/n3, so q chunks n0/n2 skip the dup DMAs.
                def go():
                    pq = pool.tile(
                        [P, 512], f32, tag="ps" if pool is psp else "aux"
                    )
                    for dc in range(DC):
                        nc.tensor.matmul(
                            pq[:],
                            w_sb[:, dc, ct * P : (ct + 1) * P],
                            x_sb[:, dc, nch * 512 : (nch + 1) * 512],
                            start=(dc == 0),
                            stop=(dc == DC - 1),
                            skip_group_check=True,
                        )
                    sl = slice(nch * 512, (nch + 1) * 512)
                    nc.vector.tensor_copy(dst[:, sl], pq[:])
                    if dup:
                        nc.sync.dma_start(dst_d[HD:P, sl], dst[0:HD, sl])
                        nc.sync.dma_start(dst_d[0:HD, sl], dst[HD:P, sl])
                return go

            def dup_chunk(dst_d, dst, nch):
                def go():
                    sl = slice(nch * 512, (nch + 1) * 512)
                    nc.sync.dma_start(dst_d[HD:P, sl], dst[0:HD, sl])
                    nc.sync.dma_start(dst_d[0:HD, sl], dst[HD:P, sl])
                return go

            def v_chunk(st, pool):
                def go():
                    pv = pool.tile(
                        [P, 512], f32, tag="ps" if pool is psp else "aux"
                    )
                    for dc in range(DC):
                        nc.tensor.matmul(
                            pv[:, 0:CL],
                            x_sb[:, dc, st * P : (st + 1) * P],
                            wv_sb[:, dc, :],
                            start=(dc == 0),
                            stop=(dc == DC - 1),
                            skip_group_check=True,
                        )
                    nc.vector.tensor_copy(
                        v_aug[:, st, :, 0:HD],
                        pv[:, 0:CL].rearrange("p (h d) -> p h d", h=HL),
                    )
                return go

            def both(f, g):
                def go():
                    f()
                    g()
                return go

            # Pre-stream: v0-v2 (overlap the wq/wk transfers), then the
            # minimum q/k operand set for pass 0 (kt<4 run serial j=1, so
            # no dups and no q chunk n1 are needed yet).
            v_chunk(0, psp)()
            v_chunk(1, psp)()
            v_chunk(2, psp)()
            proj_chunk(wq_sb, q_tiles[0], qd_tiles[0], 0, 0, psp, dup=False)()
            proj_chunk(wq_sb, q_tiles[0], qd_tiles[0], 0, 1, psp, dup=False)()
            proj_chunk(wk_sb, k_tiles[0], kd_tiles[0], 0, 0, psp, dup=False)()

            ct1_proj = [
                proj_chunk(wq_sb, q_tiles[1], qd_tiles[1], 1, 0, auxp, dup=False),
                proj_chunk(wq_sb, q_tiles[1], qd_tiles[1], 1, 1, auxp),
                proj_chunk(wq_sb, q_tiles[1], qd_tiles[1], 1, 2, auxp, dup=False),
                proj_chunk(wq_sb, q_tiles[1], qd_tiles[1], 1, 3, auxp),
                proj_chunk(wk_sb, k_tiles[1], kd_tiles[1], 1, 0, auxp),
                proj_chunk(wk_sb, k_tiles[1], kd_tiles[1], 1, 1, auxp),
                proj_chunk(wk_sb, k_tiles[1], kd_tiles[1], 1, 2, auxp),
                proj_chunk(wk_sb, k_tiles[1], kd_tiles[1], 1, 3, auxp),
            ]

            def mk_pass(hi, qp, serial_until=0, spops=None, apops=None,
                        last=False):
                ct, hh = divmod(hi, 2)
                co = hh * HD
                cx = HD - co
                return {
                    "h": hi, "qp": qp, "co": co,
                    "attnT": attnT[ct],
                    "qh": q_tiles[ct][co : co + HD, :],
                    "kh": k_tiles[ct][co : co + HD, :],
                    "qx": qd_tiles[ct][cx : cx + HD, :],
                    "kx": kd_tiles[ct][cx : cx + HD, :],
                    "serial_until": serial_until,
                    "spops": spops or {}, "apops": apops or {},
                    "last": last,
                }

            passes = [
                mk_pass(0, 0, serial_until=4,
                        spops={
                            1: dup_chunk(qd_tiles[0], q_tiles[0], 1),
                            2: both(
                                proj_chunk(wk_sb, k_tiles[0], kd_tiles[0], 0, 1, auxp),
                                dup_chunk(kd_tiles[0], k_tiles[0], 0),
                            ),
                            4: proj_chunk(wk_sb, k_tiles[0], kd_tiles[0], 0, 2, auxp),
                            7: proj_chunk(wk_sb, k_tiles[0], kd_tiles[0], 0, 3, auxp),
                            10: proj_chunk(wq_sb, q_tiles[0], qd_tiles[0], 0, 2, auxp, dup=False),
                            13: proj_chunk(wq_sb, q_tiles[0], qd_tiles[0], 0, 3, auxp),
                        },
                        apops={kt: v_chunk(kt + 2, auxp) for kt in range(1, 14)}),
                mk_pass(0, 1, apops={5: ct1_proj[0], 10: ct1_proj[1],
                                     15: ct1_proj[2]}),
                mk_pass(1, 0, apops={3: ct1_proj[3], 7: ct1_proj[4],
                                     11: ct1_proj[5], 15: ct1_proj[6]}),
                mk_pass(1, 1, apops=dict(
                    [(2, ct1_proj[7])]
                    + [(6 + i, emit_wo(0, i)) for i in range(8)])),
                mk_pass(2, 0, apops={4: emit_wo(0, 8), 7: emit_wo(0, 9),
                                     10: emit_wo(0, 10), 13: emit_wo(0, 11)}),
                mk_pass(2, 1, apops={3: emit_wo(0, 12), 7: emit_wo(0, 13),
                                     11: emit_wo(0, 14), 15: emit_wo(0, 15)}),
                mk_pass(3, 0),
                mk_pass(3, 1, apops=dict(
                    [(7, emit_wo(1, 0)), (8, emit_wo(1, 1))]
                    + [(9 + i, emit_wo(1, 2 + i, eng="act")) for i in range(6)]),
                        last=True),
            ]
            _attn_stream(nc, pools, passes, v_aug, rb_drams)

        # second half of Wo1 (tail): alternate DVE / scalar-engine PSUM
        # evacuation so neither engine paces the drain alone.
        for st in range(8, ST):
            emit_wo_wide(1, st, eng="act" if st % 2 else "dve")


def _get_nc():
    if "nc" not in _CACHE:
        _CACHE["nc"] = _build()
    return _CACHE["nc"]


def _make_in_maps(x, Wq, Wk, Wv, Wo, bo=None):
    import ml_dtypes

    qdt = ml_dtypes.bfloat16
    x = np.asarray(x)
    Wq, Wk, Wv, Wo = (np.asarray(a) for a in (Wq, Wk, Wv, Wo))
    in_maps = []
    xTs = [np.ascontiguousarray(x[b].T).astype(qdt) for b in range(x.shape[0])]
    for c in range(NCORES):
        b, g = divmod(c, GROUPS)
        sl = slice(g * CL, (g + 1) * CL)
        in_maps.append(
            {
                "xT": xTs[b],
                "wqT": np.ascontiguousarray(Wq[sl].T).astype(qdt),
                "wkT": np.ascontiguousarray(Wk[sl].T).astype(qdt),
                "wvT": np.ascontiguousarray(Wv[sl].T).astype(qdt),
                "woT": np.ascontiguousarray(Wo[:, sl].T).astype(qdt),
            }
        )
    return in_maps


def kernel(x, Wq, Wk, Wv, Wo, bo):
    x = np.asarray(x)
    bo = np.asarray(bo)
    B = x.shape[0]
    assert x.shape == (2, S, D)

    nc = _get_nc()
    in_maps = _make_in_maps(x, Wq, Wk, Wv, Wo)
    res = run_bass_kernel_spmd(nc, in_maps, core_ids=list(range(NCORES)))
    out = np.empty((B, S, D), np.float32)
    for b in range(B):
        acc = res.results[4 * b]["outp0"].astype(np.float32)
        acc = acc + res.results[4 * b]["outp1"].astype(np.float32)
        for g in range(1, GROUPS):
            acc = acc + res.results[4 * b + g]["outp0"].astype(np.float32)
            acc = acc + res.results[4 * b + g]["outp1"].astype(np.float32)
        out[b] = acc + bo[None, :]
    return out


# revision 40
# speedup vs baseline: 1.1228x; 1.1228x over previous
"""Multi-head attention (B=2, S=2048, D=1024, H=16) on 8 trn2 NeuronCores.

Sharding: data-parallel over batch (2) x tensor-parallel over head-groups (4).
Core c handles batch b = c // 4 and heads [4g, 4g+4) with g = c % 4.

Per core (all matmul operand paths bf16; PSUM accumulation fp32):
- Inputs stream in over FOUR parallel DMA queues (sync/vector/scalar/
  gpsimd issue rings) so the startup transfer wall is the largest single
  transfer, not the sum; q/k weights are split ct0/ct1 so the score
  stream's gating transfer is 0.25 MB.
- q/k are produced in transposed layout [ch, S]; v in [S, ch] layout with an
  appended ones column per head (softmax denominator rides the attnV matmul).
- Per head, scores^T = k_h @ q_h^T on the PE (K=64). Each kt-tile's two
  512-wide score matmuls are issued as a row-group pair (the second operand
  pair comes from partition-swapped copies of q/k), so both run concurrently
  in disjoint PE row groups. The first 4 units of pass 0 run the second
  half serially instead (their swapped dups don't exist yet), which lets
  the exp stream start after just one q chunk + one k chunk.
- exp on the scalar engine (1/8 scale folded in; scores are ~N(0,1) so no
  max-subtraction is needed), writing bf16 directly to SBUF.
- Attention output via the ones-augmented V (M=65) accumulated in PSUM per
  q-pass; numerators+denominator are evacuated to SBUF in bf16 (frees the
  PSUM bank early), the 1024 denominators are reshaped to [16,64] via
  SBUF-to-SBUF DMA so the DVE reciprocal uses 16 lanes instead of 1, then
  a broadcast (DRAM-bounce stride-0 normally; a K=32 PE matmul into free
  PSUM for the final tail pass) + DVE multiply produce attnT in bf16.
- Row-parallel output projection in bf16; PSUM evacuation of the late
  output tiles is split between the DVE and the (by then idle) scalar
  engine. The host upcasts and sums the 8 partials and adds the bias.

Scheduling: the attention phase is ACT(exp)-bound. All 8 (head, q-pass)
passes run as ONE continuous 128-unit stream (scores/exp DEPTH units ahead
of the in-order attnV stream) so head boundaries cost no pipeline drain.
Every projection/output tile rides inside the stream as a PE filler with
an explicit per-kt pop schedule tuned to its operand-arrival deadline.
"""

import numpy as np

import concourse.bass as bass
import concourse.tile as tile
from concourse import bacc, mybir
from concourse.bass_utils import run_bass_kernel_spmd

P = 128
S = 2048          # sequence length
D = 1024          # model dim
HD = 64           # head dim
HL = 4            # heads per core
CL = HL * HD      # 256 local channels
DC = D // P       # 8 contraction chunks
ST = S // P       # 16 seq tiles
QH = 1024         # q-half width
NCORES = 8
GROUPS = 4

f32 = mybir.dt.float32
bf16 = mybir.dt.bfloat16
EXP_BUFS = 14
OB_BUFS = 6
DEPTH = 10
FT = mybir.ActivationFunctionType

_CACHE = {}


def _attn_stream(nc, pools, passes, v_aug, rb_drams):
    """Emit all passes as one continuous unit stream. Each pass dict:
    qh/kh/qx/kx (operand row-slices), attnT, h, co, qp, serial_until
    (units below this kt emit the j=1 score matmul without row-group
    pairing), spops/apops (kt -> closure, popped in the scores / attnV
    stream), last (tail pass: PE-broadcast normalize)."""
    psp, expp, rbp, nump = pools["ps"], pools["exp"], pools["rb"], pools["num"]
    N = len(passes) * ST
    exs = {}

    def emit_scores_exp(i):
        c = passes[i // ST]
        kt = i % ST
        f = c["spops"].pop(kt, None)
        if f is not None:
            f()
        ps = psp.tile([P, QH], f32, tag="ps", name="ps")
        qoff = c["qp"] * QH
        serial = kt < c["serial_until"]
        for j in range(2):
            kt_src = c["kh"] if (j == 0 or serial) else c["kx"]
            qt_src = c["qh"] if (j == 0 or serial) else c["qx"]
            nc.tensor.matmul(
                ps[:, j * 512 : (j + 1) * 512],
                kt_src[:, kt * P : (kt + 1) * P],
                qt_src[:, qoff + j * 512 : qoff + (j + 1) * 512],
                start=True,
                stop=True,
                skip_group_check=True,
            )
        ex = expp.tile([P, QH], bf16, tag="exp", name="ex")
        nc.scalar.activation(ex[:], ps[:], FT.Exp, scale=0.125)
        exs[i] = ex

    def emit_attnv(i):
        c = passes[i // ST]
        kt = i % ST
        if kt == 0:
            c["oh"] = psp.tile([HD + 1, QH], f32, tag="ps", name="oh")
        oh = c["oh"]
        ex = exs.pop(i)
        for j in range(2):
            nc.tensor.matmul(
                oh[:, j * 512 : (j + 1) * 512],
                v_aug[:, kt, c["h"], :],
                ex[:, j * 512 : (j + 1) * 512],
                start=(kt == 0),
                stop=(kt == ST - 1),
                skip_group_check=True,
            )
        f = c["apops"].pop(kt, None)
        if f is not None:
            f()
        if kt == ST - 1:
            _normalize(i // ST, c, oh)

    def _normalize(p_idx, c, oh):
        # Evacuate numerators + denominator row to SBUF bf16 (releases the
        # oh PSUM bank after one cast), reshape the 1024 denominators to
        # [16,64] (SBUF->SBUF DMA) so the DVE reciprocal uses 16 lanes,
        # then broadcast to 64 partitions and multiply into attnT (bf16).
        co = c["co"]
        num = nump.tile([HD + 1, QH], bf16, tag="num", name="num")
        nc.vector.tensor_copy(num[:], oh[:])
        r16 = rbp.tile([16, 64], bf16, tag="r16", name="r16")
        r16b = rbp.tile([16, 64], bf16, tag="r16", name="r16b")
        nc.sync.dma_start(r16[:], num[HD : HD + 1, :])
        with nc.allow_low_precision(reason="softmax denom reciprocal bf16"):
            nc.vector.reciprocal(r16b[:], r16[:])
        dst = c["attnT"][co : co + HD, c["qp"] * QH : (c["qp"] + 1) * QH]
        if c["last"]:
            # Tail pass: scores are done, PSUM is free. Broadcast the
            # reciprocal row across 64 partitions with a K=32 PE matmul
            # (ones in row 0, zero pad below) instead of two DRAM hops.
            row_pad = pools["row_pad"]
            nc.sync.dma_start(row_pad[0:1, :], r16b[:])
            psb = psp.tile([HD, QH], f32, tag="ps", name="psb")
            for j in range(2):
                nc.tensor.matmul(
                    psb[:, j * 512 : (j + 1) * 512],
                    pools["ones_col"][:],
                    row_pad[:, j * 512 : (j + 1) * 512],
                    start=True,
                    stop=True,
                    skip_group_check=True,
                )
            nc.vector.tensor_mul(dst, num[0:HD, :], psb[:])
            return
        rbd_b = rb_drams[p_idx]
        w2 = nc.sync.dma_start(
            rbd_b[:].rearrange("a (p f) -> (a p) f", p=16), r16b[:]
        )
        rbt = rbp.tile([HD, QH], bf16, tag="rbt", name="rbt")
        dram_ap = rbd_b[0:1, :]
        bcast_src = bass.AP(
            tensor=dram_ap.tensor,
            offset=dram_ap.offset,
            ap=[[0, HD]] + list(dram_ap.ap[1:]),
        )
        r2 = nc.gpsimd.dma_start(rbt[:], bcast_src)
        tile.add_dep_helper(r2.ins, w2.ins, sync=True, reason="recip bounce")
        nc.vector.tensor_mul(dst, num[0:HD, :], rbt[:])

    for i in range(N + DEPTH):
        if i < N:
            emit_scores_exp(i)
        if i >= DEPTH:
            emit_attnv(i - DEPTH)
    for c in passes:
        assert not c["spops"] and not c["apops"], "unconsumed fillers"


def _build(reps=1):
    nc = bacc.Bacc(None, target_bir_lowering=False)
    xT = nc.dram_tensor("xT", [D, S], bf16, kind="ExternalInput")
    wqT = nc.dram_tensor("wqT", [D, CL], bf16, kind="ExternalInput")
    wkT = nc.dram_tensor("wkT", [D, CL], bf16, kind="ExternalInput")
    wvT = nc.dram_tensor("wvT", [D, CL], bf16, kind="ExternalInput")
    woT = nc.dram_tensor("woT", [CL, D], bf16, kind="ExternalInput")
    outs = [
        nc.dram_tensor(f"outp{j}", [S, D], bf16, kind="ExternalOutput")
        for j in range(2)
    ]

    with tile.TileContext(nc) as tc:
        for rep in range(reps):
            if rep:
                tc.strict_bb_all_engine_barrier()
            _emit_body(nc, tc, xT, wqT, wkT, wvT, woT, outs, rep)
    nc.compile()
    return nc


def _emit_body(nc, tc, xT, wqT, wkT, wvT, woT, outs, rep):
    rb_drams = [
        nc.dram_tensor(f"rbd_{rep}_{i}", [1, QH], bf16, kind="Internal")
        for i in range(8)
    ]
    with (
        tc.tile_pool(name="main", bufs=1) as main,
        tc.tile_pool(name="qk", bufs=2) as qkp,
        tc.tile_pool(name="exp", bufs=EXP_BUFS) as expp,
        tc.tile_pool(name="rb", bufs=2) as rbp,
        tc.tile_pool(name="num", bufs=2) as nump,
        tc.tile_pool(name="ob", bufs=OB_BUFS) as obp,
        tc.tile_pool(name="ps", bufs=3, space="PSUM") as psp,
        tc.tile_pool(name="aux", bufs=2, space="PSUM") as auxp,
    ):
        pools = {"ps": psp, "exp": expp, "rb": rbp, "num": nump}

        v_aug = main.tile([P, ST, HL, HD + 1], bf16)
        nc.vector.memset(v_aug[:, :, :, HD : HD + 1], 1.0)
        # K=32 broadcast operands: stationary has ones in row 0 and zeros
        # below; the moving row-pad is zeroed so uninit rows can't inject
        # NaN (anything x 0 weight must be a real 0 contribution).
        ones_col = main.tile([32, HD], bf16)
        nc.vector.memset(ones_col[:], 0.0)
        nc.vector.memset(ones_col[0:1, :], 1.0)
        row_pad = main.tile([32, QH], bf16)
        nc.vector.memset(row_pad[:], 0.0)
        pools["ones_col"] = ones_col
        pools["row_pad"] = row_pad
        # touch Exp once so the ACT table load happens during the DMA
        # head instead of delaying the first real softmax exp
        actwarm = main.tile([P, 1], f32)
        nc.vector.memset(actwarm[:], 1.0)
        nc.scalar.activation(actwarm[:], actwarm[:], FT.Exp)
        attnT0 = main.tile([P, S], bf16, tag="attnT0")
        attnT1 = main.tile([P, S], bf16, tag="attnT1")
        attnT = [attnT0, attnT1]
        wo_sb = main.tile([P, 2, D], bf16)

        def wo_evac(ob, po, eng):
            # PSUM evacuation: DVE normally; the scalar engine once the
            # exp stream is done (it can read PSUM too).
            if eng == "act":
                nc.scalar.copy(ob[:], po[:])
            else:
                nc.vector.tensor_copy(ob[:], po[:])

        def emit_wo(cc, st, eng="dve"):
            def go():
                ob = obp.tile([P, QH], bf16, tag="obw")
                for j in range(2):
                    po = auxp.tile([P, 512], f32, tag="aux")
                    nc.tensor.matmul(
                        po[:],
                        attnT[cc][:, st * P : (st + 1) * P],
                        wo_sb[:, cc, j * 512 : (j + 1) * 512],
                        start=True,
                        stop=True,
                        skip_group_check=True,
                    )
                    wo_evac(ob[:, j * 512 : (j + 1) * 512], po, eng)
                nc.gpsimd.dma_start(outs[cc][st * P : (st + 1) * P, :], ob[:])
            return go

        def emit_wo_wide(cc, st, eng="dve"):
            # Tail variant: scores are done, borrow a [P, 1024] PSUM tile
            # from the ps pool, one fused evacuation, one output DMA.
            po = psp.tile([P, QH], f32, tag="ps")
            for j in range(2):
                nc.tensor.matmul(
                    po[:, j * 512 : (j + 1) * 512],
                    attnT[cc][:, st * P : (st + 1) * P],
                    wo_sb[:, cc, j * 512 : (j + 1) * 512],
                    start=True,
                    stop=True,
                    skip_group_check=True,
                )
            ob = obp.tile([P, QH], bf16, tag="obw")
            wo_evac(ob, po, eng)
            nc.gpsimd.dma_start(outs[cc][st * P : (st + 1) * P, :], ob[:])

        with tc.tile_pool(name="w", bufs=1) as wp:
            x_sb = wp.tile([P, DC, S], bf16)
            wq_sb = wp.tile([P, DC, CL], bf16)
            wk_sb = wp.tile([P, DC, CL], bf16)
            wv_sb = wp.tile([P, DC, CL], bf16)
            # Parallel DMA head across four issue queues: the stream is
            # gated on x block 0 + wq ct0 + wk ct0 (+ wv for v tiles), so
            # those go on separate queues and the ct1 halves come later.
            xTr = xT[:].rearrange("(c p) s -> p c s", p=P)
            wqTr = wqT[:].rearrange("(c p) m -> p c m", p=P)
            wkTr = wkT[:].rearrange("(c p) m -> p c m", p=P)
            nc.sync.dma_start(x_sb[:, :, 0:256], xTr[:, :, 0:256])
            nc.gpsimd.dma_start(x_sb[:, :, 256:512], xTr[:, :, 256:512])
            nc.scalar.dma_start(
                wv_sb[:], wvT[:].rearrange("(c p) m -> p c m", p=P)
            )
            nc.scalar.dma_start(wq_sb[:, :, 0:P], wqTr[:, :, 0:P])
            nc.scalar.dma_start(wk_sb[:, :, 0:P], wkTr[:, :, 0:P])
            nc.gpsimd.dma_start(x_sb[:, :, 512:1024], xTr[:, :, 512:1024])
            nc.scalar.dma_start(x_sb[:, :, 1024:1536], xTr[:, :, 1024:1536])
            nc.scalar.dma_start(x_sb[:, :, 1536:2048], xTr[:, :, 1536:2048])
            nc.gpsimd.dma_start(wq_sb[:, :, P:CL], wqTr[:, :, P:CL])
            nc.gpsimd.dma_start(wk_sb[:, :, P:CL], wkTr[:, :, P:CL])
            nc.gpsimd.dma_start(
                wo_sb[:], woT[:].rearrange("(c p) d -> p c d", p=P)
            )

            # HAM warm-up: the PE idles through the barrier + DMA head and
            # re-throttles to 1.2 GHz; ~8us of junk matmuls on the zeroed
            # row_pad keep it busy so the pre-stream projections (and the
            # first exp) run at full clock.
            warm = psp.tile([HD, 512], f32, tag="ps", name="warm")
            for _ in range(10):
                nc.tensor.matmul(
                    warm[:],
                    ones_col[:],
                    row_pad[:, 0:512],
                    start=True,
                    stop=True,
                    skip_group_check=True,
                )

            q_tiles, k_tiles, qd_tiles, kd_tiles = [], [], [], []
            for ct in range(2):
                q_tiles.append(qkp.tile([P, S], bf16, tag="q", name=f"q{ct}"))
                k_tiles.append(qkp.tile([P, S], bf16, tag="k", name=f"k{ct}"))
                qd_tiles.append(qkp.tile([P, S], bf16, tag="qd", name=f"qd{ct}"))
                kd_tiles.append(qkp.tile([P, S], bf16, tag="kd", name=f"kd{ct}"))

            def proj_chunk(w_sb, dst, dst_d, ct, nch, pool, dup=True):
                # q/k projection chunk: [P, 512] of the transposed q/k,
                # plus (optionally) the partition-swapped dup used by the
                # row-group-paired j=1 score matmuls. qd is only ever read
                # at chunks n1/n3, so q chunks n0/n2 skip the dup DMAs.
                def go():
                    pq = pool.tile(
                        [P, 512], f32, tag="ps" if pool is psp else "aux"
                    )
                    for dc in range(DC):
                        nc.tensor.matmul(
                            pq[:],
                            w_sb[:, dc, ct * P : (ct + 1) * P],
                            x_sb[:, dc, nch * 512 : (nch + 1) * 512],
                            start=(dc == 0),
                            stop=(dc == DC - 1),
                            skip_group_check=True,
                        )
                    sl = slice(nch * 512, (nch + 1) * 512)
                    nc.vector.tensor_copy(dst[:, sl], pq[:])
                    if dup:
                        nc.sync.dma_start(dst_d[HD:P, sl], dst[0:HD, sl])
                        nc.sync.dma_start(dst_d[0:HD, sl], dst[HD:P, sl])
                return go

            def dup_chunk(dst_d, dst, nch):
                def go():
                    sl = slice(nch * 512, (nch + 1) * 512)
                    nc.sync.dma_start(dst_d[HD:P, sl], dst[0:HD, sl])
                    nc.sync.dma_start(dst_d[0:HD, sl], dst[HD:P, sl])
                return go

            def v_chunk(st, pool):
                def go():
                    pv = pool.tile(
                        [P, 512], f32, tag="ps" if pool is psp else "aux"
                    )
                    for dc in range(DC):
                        nc.tensor.matmul(
                            pv[:, 0:CL],
                            x_sb[:, dc, st * P : (st + 1) * P],
                            wv_sb[:, dc, :],
                            start=(dc == 0),
                            stop=(dc == DC - 1),
                            skip_group_check=True,
                        )
                    nc.vector.tensor_copy(
                        v_aug[:, st, :, 0:HD],
                        pv[:, 0:CL].rearrange("p (h d) -> p h d", h=HL),
                    )
                return go

            def both(f, g):
                def go():
                    f()
                    g()
                return go

            # Pre-stream: v0-v2 (overlap the wq/wk transfers), then the
            # minimum q/k operand set for pass 0 (kt<4 run serial j=1, so
            # no dups and no q chunk n1 are needed yet).
            v_chunk(0, psp)()
            v_chunk(1, psp)()
            v_chunk(2, psp)()
            proj_chunk(wq_sb, q_tiles[0], qd_tiles[0], 0, 0, psp, dup=False)()
            proj_chunk(wq_sb, q_tiles[0], qd_tiles[0], 0, 1, psp, dup=False)()
            proj_chunk(wk_sb, k_tiles[0], kd_tiles[0], 0, 0, psp, dup=False)()

            ct1_proj = [
                proj_chunk(wq_sb, q_tiles[1], qd_tiles[1], 1, 0, auxp, dup=False),
                proj_chunk(wq_sb, q_tiles[1], qd_tiles[1], 1, 1, auxp),
                proj_chunk(wq_sb, q_tiles[1], qd_tiles[1], 1, 2, auxp, dup=False),
                proj_chunk(wq_sb, q_tiles[1], qd_tiles[1], 1, 3, auxp),
                proj_chunk(wk_sb, k_tiles[1], kd_tiles[1], 1, 0, auxp),
                proj_chunk(wk_sb, k_tiles[1], kd_tiles[1], 1, 1, auxp),
                proj_chunk(wk_sb, k_tiles[1], kd_tiles[1], 1, 2, auxp),
                proj_chunk(wk_sb, k_tiles[1], kd_tiles[1], 1, 3, auxp),
            ]

            def mk_pass(hi, qp, serial_until=0, spops=None, apops=None,
                        last=False):
                ct, hh = divmod(hi, 2)
                co = hh * HD
                cx = HD - co
                return {
                    "h": hi, "qp": qp, "co": co,
                    "attnT": attnT[ct],
                    "qh": q_tiles[ct][co : co + HD, :],
                    "kh": k_tiles[ct][co : co + HD, :],
                    "qx": qd_tiles[ct][cx : cx + HD, :],
                    "kx": kd_tiles[ct][cx : cx + HD, :],
                    "serial_until": serial_until,
                    "spops": spops or {}, "apops": apops or {},
                    "last": last,
                }

            passes = [
                mk_pass(0, 0, serial_until=4,
                        spops={
                            1: dup_chunk(qd_tiles[0], q_tiles[0], 1),
                            2: both(
                                proj_chunk(wk_sb, k_tiles[0], kd_tiles[0], 0, 1, auxp),
                                dup_chunk(kd_tiles[0], k_tiles[0], 0),
                            ),
                            4: proj_chunk(wk_sb, k_tiles[0], kd_tiles[0], 0, 2, auxp),
                            7: proj_chunk(wk_sb, k_tiles[0], kd_tiles[0], 0, 3, auxp),
                            10: proj_chunk(wq_sb, q_tiles[0], qd_tiles[0], 0, 2, auxp, dup=False),
                            13: proj_chunk(wq_sb, q_tiles[0], qd_tiles[0], 0, 3, auxp),
                        },
                        apops={kt: v_chunk(kt + 2, auxp) for kt in range(1, 14)}),
                mk_pass(0, 1, apops={5: ct1_proj[0], 10: ct1_proj[1],
                                     15: ct1_proj[2]}),
                mk_pass(1, 0, apops={3: ct1_proj[3], 7: ct1_proj[4],
                                     11: ct1_proj[5], 15: ct1_proj[6]}),
                mk_pass(1, 1, apops=dict(
                    [(2, ct1_proj[7])]
                    + [(8 + i, emit_wo(0, i)) for i in range(8)])),
                mk_pass(2, 0, apops={6: emit_wo(0, 8), 9: emit_wo(0, 9),
                                     12: emit_wo(0, 10), 15: emit_wo(0, 11)}),
                mk_pass(2, 1, apops={3: emit_wo(0, 12), 7: emit_wo(0, 13),
                                     11: emit_wo(0, 14), 15: emit_wo(0, 15)}),
                mk_pass(3, 0),
                mk_pass(3, 1, apops=dict(
                    [(8, emit_wo(1, 0)), (9, emit_wo(1, 1))]
                    + [(10 + i, emit_wo(1, 2 + i, eng="act")) for i in range(6)]),
                        last=True),
            ]
            _attn_stream(nc, pools, passes, v_aug, rb_drams)

        # second half of Wo1 (tail): alternate DVE / scalar-engine PSUM
        # evacuation so neither engine paces the drain alone.
        for st in range(8, ST):
            emit_wo_wide(1, st, eng="act" if st % 2 else "dve")


def _get_nc():
    if "nc" not in _CACHE:
        _CACHE["nc"] = _build()
    return _CACHE["nc"]


def _make_in_maps(x, Wq, Wk, Wv, Wo, bo=None):
    import ml_dtypes

    qdt = ml_dtypes.bfloat16
    x = np.asarray(x)
    Wq, Wk, Wv, Wo = (np.asarray(a) for a in (Wq, Wk, Wv, Wo))
    in_maps = []
    xTs = [np.ascontiguousarray(x[b].T).astype(qdt) for b in range(x.shape[0])]
    for c in range(NCORES):
        b, g = divmod(c, GROUPS)
        sl = slice(g * CL, (g + 1) * CL)
        in_maps.append(
            {
                "xT": xTs[b],
                "wqT": np.ascontiguousarray(Wq[sl].T).astype(qdt),
                "wkT": np.ascontiguousarray(Wk[sl].T).astype(qdt),
                "wvT": np.ascontiguousarray(Wv[sl].T).astype(qdt),
                "woT": np.ascontiguousarray(Wo[:, sl].T).astype(qdt),
            }
        )
    return in_maps


def kernel(x, Wq, Wk, Wv, Wo, bo):
    x = np.asarray(x)
    bo = np.asarray(bo)
    B = x.shape[0]
    assert x.shape == (2, S, D)

    nc = _get_nc()
    in_maps = _make_in_maps(x, Wq, Wk, Wv, Wo)
    res = run_bass_kernel_spmd(nc, in_maps, core_ids=list(range(NCORES)))
    out = np.empty((B, S, D), np.float32)
    for b in range(B):
        acc = res.results[4 * b]["outp0"].astype(np.float32)
        acc = acc + res.results[4 * b]["outp1"].astype(np.float32)
        for g in range(1, GROUPS):
            acc = acc + res.results[4 * b + g]["outp0"].astype(np.float32)
            acc = acc + res.results[4 * b + g]["outp1"].astype(np.float32)
        out[b] = acc + bo[None, :]
    return out
